# revision 54
# baseline (speedup 1.0000x reference)
"""Trainium2 Bass kernel for nn_PostAttention (sparse_attention).

Computation (B=1, N=4096, H=8, d_qk=96, d_v=64):
    proj = qk @ W_qk -> q, k per head;  v = v_cls @ W_v per head
    S = q @ k.T * scale;  E = exp(S);  Z_i = sum_j E
    out_i = sum_j E_ij * m_ij * v_j / (Z_i * H * M_i),  M_i = sum_j m_ij

Sharding: 8 cores as 2 query-row groups x 4 head groups (2 heads/core).

Division of labor vs the device:
  * host pre-scales the mask rows by 2^14/(H*M_i)  -> no M column sums and
    no per-query normalization math on-chip
  * host divides the returned numerator O by (Z * 2^14) -> no PE transposes,
    no reciprocal chain; the device returns O^T ([e, i]) and Z rows as-is.

Device layout: everything transposed (S^T = [key j on partitions, query i on
free dim]) so exp output E^T / P^T feed the P@V matmul directly as the moving
operand.  exp runs on ScalarE over PAIRS of j-tiles ([128, 2*512] spanning two
PSUM banks); back-to-back ACTIVATEs pipeline to ~N/1.2GHz, making ScalarE the
pacing engine for the attention phase.  To keep it busy from the start, the
i-chunk 0 attention stream is interleaved into the projection phase (pair p
of i-chunk 0 only needs the KT/V chunks produced up to projection chunk p//2),
the Q projections for i-chunks 1-3 are deferred into the attention stream
(their qk chunks stay resident in SBUF), and one global consumer pipeline
runs across i-chunk boundaries.  Z column sums are ones-vector matmuls packed
into spare PE column groups; both heads' Z streams share one PSUM bank and
both heads' PV streams share one PSUM bank (the has_written clear from
start=True is scoped to the partition range the matmul's column groups
engage, so per-head streams with their own start=True don't disturb each
other).  PSUM budget: 2 proj accumulators + 4 (phase A) / 6 (phase B) S
banks + Z + O = 8.

dtypes: fp16 operands everywhere on the PE (1 cyc/row), fp32 PSUM accum.
(fp8 was tried and rejected: for signed contractions the per-element e4m3
quantization error does NOT average down -- measured 9.9e-2 output error.)
"""
import os
import sys

sys.path.insert(0, "/opt/trn_rl_repo")
import numpy as np

import concourse.mybir as mybir
import concourse.tile as tile
from concourse import bacc
from concourse.bass_utils import run_bass_kernel_spmd

f32 = mybir.dt.float32
f16 = mybir.dt.float16
FT = mybir.ActivationFunctionType

N = 4096
H = 8
DQK = 96
DV = 64
R = 2              # row groups
C = 4              # head groups
HPC = H // C       # heads per core = 2
NQ = N // R        # queries per core = 2048
NIC = NQ // 512    # i-chunks per core = 4
NJT = N // 128     # j tiles = 32
NPAIR = NJT // 2   # j-tile pairs = 16
SCALE = (256 // 8) ** -0.5
EXP_BIAS = -4.0    # uniform shift inside exp; cancels in the Z ratio
MASK_SHIFT = 14    # mask rows pre-scaled by 2^14/(H*M_i); host divides it out

_CACHED = {}


def _build_nc():
    nc = bacc.Bacc(name="post_attention")

    qkT = nc.declare_dram_parameter("qkT", [8, 128, 6, 512], f16, isOutput=False)
    vT = nc.declare_dram_parameter("vT", [8, 128, 4, 512], f16, isOutput=False)
    wq = nc.declare_dram_parameter("wq", [128, 6, HPC * DQK], f16, isOutput=False)
    wk = nc.declare_dram_parameter("wk", [128, 6, HPC * DQK], f16, isOutput=False)
    wv = nc.declare_dram_parameter("wv", [128, 4, HPC * DV], f16, isOutput=False)
    maskt = nc.declare_dram_parameter("maskt", [NIC, NPAIR, 128, 2, 512], f16, isOutput=False)
    # out[ic] = O^T chunk [e(2 heads x 64), i(512)]; zout[ic] = Z rows [h, i]
    out = nc.declare_dram_parameter("out", [NIC, 128, 512], f32, isOutput=True)
    zout = nc.declare_dram_parameter("zout", [NIC, HPC, 512], f32, isOutput=True)

    with tile.TileContext(nc) as tc:
        with (
            tc.tile_pool(name="const", bufs=1) as const,
            tc.tile_pool(name="persist", bufs=1) as persist,
            tc.tile_pool(name="mt", bufs=3) as mtp,
            tc.tile_pool(name="ep", bufs=3) as ep,
            tc.tile_pool(name="fin", bufs=2) as fin,
            tc.tile_pool(name="ps_z", bufs=1, space="PSUM") as ps_z,
            tc.tile_pool(name="ps_o", bufs=1, space="PSUM") as ps_o,
        ):
            ones16 = const.tile([128, 1], f16)
            nc.vector.memset(ones16, 1.0)
            bias_t = const.tile([128, 1], f32)
            nc.vector.memset(bias_t, EXP_BIAS)

            QT = persist.tile([DQK, HPC, NQ], f16)
            KT = persist.tile([DQK, HPC, N], f16)
            V = persist.tile([128, NJT, HPC * DV], f16)
            # qk chunks 1..3 stay resident so the Q projections for
            # i-chunks 1..3 can run in phase B (hidden under the exp pace)
            # instead of lengthening the PE-bound phase A.
            qkeep = persist.tile([128, 3, 6, 512], f16)

            # One compiled kernel serves all cores: the host rolls the N axis
            # of qkT / vT / mask columns so this core's query rows sit at
            # columns [0, NQ); the j-sum is order-invariant.

            def attn_unit(ic, jt0, njt, s_tile_fn):
                """Emit mask DMA + S matmuls + exp + mask-mul for j-tiles
                jt0..jt0+njt-1 of i-chunk ic.  Returns [(e_t, p_t)] per
                head.  Bigger njt amortizes the ~190-cycle fixed cost of
                each ACTIVATE on the pacing engine."""
                icol = slice(ic * 512, (ic + 1) * 512)
                mt = mtp.tile([128, njt, 512], f16, tag="mt", name="mt")
                nc.sync.dma_start(out=mt, in_=maskt[ic, jt0 // 2])
                cur = []
                for h in range(HPC):
                    s_ps = s_tile_fn(h, njt)
                    for t in range(njt):
                        jrow = slice((jt0 + t) * 128, (jt0 + t + 1) * 128)
                        nc.tensor.matmul(
                            s_ps[:, t, :],
                            lhsT=KT[:, h, jrow],
                            rhs=QT[:, h, icol],
                        )
                    e_t = ep.tile([128, njt, 512], f16, tag=f"e{h}", name=f"e_t{h}")
                    nc.scalar.activation(e_t, s_ps, FT.Exp, bias=bias_t, scale=SCALE)
                    p_t = ep.tile([128, njt, 512], f16, tag=f"p{h}", name=f"p_t{h}")
                    nc.vector.tensor_mul(p_t, e_t, mt)
                    cur.append((e_t, p_t))
                return cur

            def attn_consumers(jt0, njt, prev, z_ps, o_ps):
                """Z and PV matmuls for one unit (inputs `prev` from
                attn_unit).  Both heads' streams share one Z bank / one O
                bank; col-group pairs run concurrently on the PE.
                (fp8 DoubleRow can't be used here: it requires the matmul
                dst to start at partition 0, which breaks the col-group
                packing that makes the two heads concurrent.)"""
                for t in range(njt):
                    first = jt0 + t == 0
                    last = jt0 + t == NJT - 1
                    for h in range(HPC):
                        nc.tensor.matmul(
                            z_ps[32 * h : 32 * h + 1, :],
                            lhsT=ones16,
                            rhs=prev[h][0][:, t, :],
                            start=first,
                            stop=last,
                            tile_position=(0, 32 * h),
                            skip_group_check=True,
                        )
                for t in range(njt):
                    jt = jt0 + t
                    first = jt == 0
                    last = jt == NJT - 1
                    for h in range(HPC):
                        nc.tensor.matmul(
                            o_ps[64 * h : 64 * (h + 1), :],
                            lhsT=V[:, jt, 64 * h : 64 * (h + 1)],
                            rhs=prev[h][1][:, t, :],
                            start=first,
                            stop=last,
                            tile_position=(0, 64 * h),
                            skip_group_check=True,
                        )

            def drain_ic(ic, z_ps, o_ps):
                # Z first: the next i-chunk's consumers hit the Z bank
                # before the O bank.  The out DMA splits 4 ways so its
                # 2KB-per-partition descriptors spread across queues.
                z_sb = fin.tile([33, 512], f32, tag="zsb", name="z_sb")
                for h in range(HPC):
                    nc.vector.tensor_copy(z_sb[32 * h : 32 * h + 1, :], z_ps[32 * h : 32 * h + 1, :])
                    nc.sync.dma_start(out=zout[ic, h : h + 1, :], in_=z_sb[32 * h : 32 * h + 1, :])
                o_sb = fin.tile([128, 512], f32, tag="osb", name="o_sb")
                nc.vector.tensor_copy(o_sb, o_ps)
                for q in range(4):
                    sl = slice(32 * q, 32 * (q + 1))
                    nc.sync.dma_start(out=out[ic, sl], in_=o_sb[sl])

            # One global software pipeline across all (ic, unit) tiles:
            # consumers lag the S/exp/mul stream by two units, INCLUDING
            # across i-chunk boundaries, so the exp engine never drains while
            # the PE runs an epilogue.
            zo = {}
            pending = []  # [(ic, jt0, njt, cur), ...] awaiting consumers

            def push_unit(ic, jt0, njt, cur):
                pending.append((ic, jt0, njt, cur))
                if len(pending) > 2:
                    flush_unit()

            def flush_unit():
                icj, jt0, njt, prev = pending.pop(0)
                attn_consumers(jt0, njt, prev, *zo[icj])
                if jt0 + njt == NJT:
                    drain_ic(icj, *zo[icj])

            # ---- phase A: projection with i-chunk 0 attention interleaved ----
            # PSUM: proj accumulators 2 banks (rotating tag) + S pairs for
            # ic0 2x2 banks (tag per head) + Z 1 + O 1 = 8.
            zo[0] = (
                ps_z.tile([33, 512], f32, tag="z", name="z_ps0"),
                ps_o.tile([128, 512], f32, tag="o", name="o_ps0"),
            )
            with (
                tc.tile_pool(name="wpool", bufs=1) as wpool,
                tc.tile_pool(name="qs", bufs=3) as qs,
                tc.tile_pool(name="pp", bufs=2, space="PSUM") as pp,
                tc.tile_pool(name="ps_sa", bufs=1, space="PSUM") as ps_sa,
            ):
                wq_t = persist.tile([128, 6, HPC * DQK], f16)  # outlives phase A
                nc.sync.dma_start(out=wq_t, in_=wq[:, :, :])
                wk_t = wpool.tile([128, 6, HPC * DQK], f16)
                nc.sync.dma_start(out=wk_t, in_=wk[:, :, :])
                wv_t = wpool.tile([128, 4, HPC * DV], f16)
                nc.sync.dma_start(out=wv_t, in_=wv[:, :, :])

                np0 = 0  # pairs emitted for ic0

                def ic0_step():
                    nonlocal np0
                    cur = attn_unit(
                        0, 2 * np0, 2,
                        lambda h, njt: ps_sa.tile([128, njt, 512], f32, tag=f"s{h}", name=f"sa_ps{h}"),
                    )
                    push_unit(0, 2 * np0, 2, cur)
                    np0 += 1

                for n in range(8):
                    ncol = slice(n * 512, (n + 1) * 512)
                    if 1 <= n <= 3:
                        qk_sl = qkeep[:, n - 1]
                    else:
                        qk_sl = qs.tile([128, 6, 512], f16, tag="qksl", name="qk_sl")
                    # host pre-tiles qkT/vT per chunk: each DMA is one
                    # contiguous 3KB/2KB line per partition (descriptor-rate
                    # bound otherwise).  Chunk 0 gates the kernel start, so
                    # it splits per-c across six queues.
                    if n == 0:
                        for c in range(6):
                            nc.sync.dma_start(out=qk_sl[:, c, :], in_=qkT[n, :, c, :])
                    else:
                        nc.sync.dma_start(out=qk_sl[:, 0:3, :], in_=qkT[n, :, 0:3, :])
                        nc.sync.dma_start(out=qk_sl[:, 3:6, :], in_=qkT[n, :, 3:6, :])
                    v_sl = qs.tile([128, 4, 512], f16, tag="vsl")
                    nc.sync.dma_start(out=v_sl, in_=vT[n])

                    for h in range(HPC):
                        kt_ps = pp.tile([DQK, 512], f32, tag="pj", name=f"kt_ps{h}")
                        for c in range(6):
                            nc.tensor.matmul(
                                kt_ps,
                                lhsT=wk_t[:, c, h * DQK : (h + 1) * DQK],
                                rhs=qk_sl[:, c, :],
                                start=(c == 0),
                                stop=(c == 5),
                            )
                        nc.vector.tensor_copy(KT[:, h, ncol], kt_ps)

                    if n == 0:  # query rows live in columns [0, NQ) after host roll
                        for h in range(HPC):
                            qt_ps = pp.tile([DQK, 512], f32, tag="pj", name=f"qt_ps{h}")
                            for c in range(6):
                                nc.tensor.matmul(
                                    qt_ps,
                                    lhsT=wq_t[:, c, h * DQK : (h + 1) * DQK],
                                    rhs=qk_sl[:, c, :],
                                    start=(c == 0),
                                    stop=(c == 5),
                                )
                            nc.vector.tensor_copy(QT[:, h, ncol], qt_ps)

                    # V directly in [j, e] layout: contraction over channel
                    # chunks with vT slices as the stationary operand.
                    for jj in range(4):
                        jt = 4 * n + jj
                        vj_ps = pp.tile([128, HPC * DV], f32, tag="pj", name="vj_ps")
                        for c in range(4):
                            nc.tensor.matmul(
                                vj_ps,
                                lhsT=v_sl[:, c, jj * 128 : (jj + 1) * 128],
                                rhs=wv_t[:, c, :],
                                start=(c == 0),
                                stop=(c == 3),
                            )
                        nc.vector.tensor_copy(V[:, jt, :], vj_ps)

                    # interleave i-chunk 0 attention: pairs 2n, 2n+1 only
                    # need KT/V chunks <= n (with the depth-2 consumer lag).
                    if n >= 1:
                        ic0_step()
                        ic0_step()

                def qproj(icq, qt_tile_fn):
                    """Q projection for i-chunk icq, emitted where the PE has
                    slack (the exp engine is saturated with backlog); the
                    PSUM accumulator borrows a slot of the local S rotation."""
                    icolq = slice(icq * 512, (icq + 1) * 512)
                    for h in range(HPC):
                        qt_ps = qt_tile_fn(h)
                        for c in range(6):
                            nc.tensor.matmul(
                                qt_ps,
                                lhsT=wq_t[:, c, h * DQK : (h + 1) * DQK],
                                rhs=qkeep[:, icq - 1, c, :],
                                start=(c == 0),
                                stop=(c == 5),
                            )
                        nc.vector.tensor_copy(QT[:, h, icolq], qt_ps)

                # pp's banks are idle once chunk 7 is done; the exp engine
                # still has a pair backlog here, so the PE has slack.
                qproj(1, lambda h: pp.tile([DQK, 512], f32, tag="pj", name=f"qt_ps{h}"))
                while np0 < NPAIR:
                    ic0_step()

            # ---- phase B: attention i-chunks 1..3 ----
            # ic0's last two consumer groups flush inside the global loop,
            # after phase B's first S units, so ScalarE never idles at the
            # phase seam or at i-chunk boundaries.  Each i-chunk's Q
            # projection runs here too (hidden under the exp pace), its PSUM
            # accumulator borrowing a slot of the S pool rotation.
            with tc.tile_pool(name="ps_sb", bufs=3, space="PSUM") as ps_sb:
                for ic in range(1, NIC):
                    zo[ic] = (
                        ps_z.tile([33, 512], f32, tag="z", name="z_ps"),
                        ps_o.tile([128, 512], f32, tag="o", name="o_ps"),
                    )
                    for p in range(NPAIR):
                        if p == 8 and ic < NIC - 1:
                            qproj(ic + 1, lambda h: ps_sb.tile([DQK, 512], f32, tag="s", name=f"qt_ps{h}"))
                        cur = attn_unit(
                            ic, 2 * p, 2,
                            lambda h, njt: ps_sb.tile([128, njt, 512], f32, tag="s", name=f"s_ps{h}"),
                        )
                        push_unit(ic, 2 * p, 2, cur)
                while pending:
                    flush_unit()

    nc.finalize()
    return nc


def kernel(**inputs) -> np.ndarray:
    qk = np.asarray(inputs["qk"], dtype=np.float32)        # [1, N, 768]
    v_cls = np.asarray(inputs["v_cls"], dtype=np.float32)  # [1, N, 512]
    masks = np.asarray(inputs["masks"], dtype=np.float32)  # [1, N, N]
    W_qk = np.asarray(inputs["W_qk"], dtype=np.float32)    # [768, 1536]
    W_v = np.asarray(inputs["W_v"], dtype=np.float32)      # [512, 512]

    if "nc" not in _CACHED:
        _CACHED["nc"] = _build_nc()
    nc = _CACHED["nc"]

    # Pre-scale mask rows by 2^14/(H*M_i): folds the H and mask-sum
    # normalizations into the mask; the host divides the 2^14 back out.
    m0 = masks[0]
    row_w = (2.0 ** MASK_SHIFT) / (H * m0.sum(axis=1))
    mask_scaled = (m0 * row_w[:, None]).astype(np.float16)
    # Roll the key/value axis per row group so each core's query rows start at
    # column 0; the kernel reads Q from columns [0, NQ) and pairs KT j-tiles
    # with identically rolled mask columns, so the j-sum is just reordered.
    # All device inputs are pre-tiled so every DMA reads one contiguous
    # line per partition (the DMA engines are descriptor-rate bound).
    qkT_rg, vT_rg, mask_rg = [], [], []
    for rg in range(R):
        h0 = rg * NQ
        qk_roll = np.roll(qk[0], -h0, axis=0)
        v_roll = np.roll(v_cls[0], -h0, axis=0)
        qkT_rg.append(np.ascontiguousarray(
            qk_roll.T.astype(np.float16).reshape(6, 128, 8, 512).transpose(2, 1, 0, 3)))
        vT_rg.append(np.ascontiguousarray(
            v_roll.T.astype(np.float16).reshape(4, 128, 8, 512).transpose(2, 1, 0, 3)))
        mT = np.roll(mask_scaled[h0 : h0 + NQ], -h0, axis=1).T
        mask_rg.append(np.ascontiguousarray(
            mT.reshape(NPAIR, 2, 128, NIC, 512).transpose(3, 0, 2, 1, 4)))
    wq_hg, wk_hg, wv_hg = [], [], []
    for hg in range(C):
        hs = hg * HPC
        wq_hg.append(np.ascontiguousarray(
            W_qk[:, hs * DQK : (hs + HPC) * DQK].astype(np.float16).reshape(6, 128, HPC * DQK).transpose(1, 0, 2)))
        wk_hg.append(np.ascontiguousarray(
            W_qk[:, 768 + hs * DQK : 768 + (hs + HPC) * DQK].astype(np.float16).reshape(6, 128, HPC * DQK).transpose(1, 0, 2)))
        wv_hg.append(np.ascontiguousarray(
            W_v[:, hs * DV : (hs + HPC) * DV].astype(np.float16).reshape(4, 128, HPC * DV).transpose(1, 0, 2)))
    in_maps = []
    for core in range(8):
        rg, hg = divmod(core, C)
        in_maps.append({
            "qkT": qkT_rg[rg],
            "vT": vT_rg[rg],
            "wq": wq_hg[hg],
            "wk": wk_hg[hg],
            "wv": wv_hg[hg],
            "maskt": mask_rg[rg],
        })

    trace = os.environ.get("KERNEL_TRACE", "0") == "1"
    res = run_bass_kernel_spmd(nc, in_maps, list(range(8)), trace=trace)
    if trace:
        _CACHED["exec_time_ns"] = res.exec_time_ns
        _CACHED["mean_exec_time_ns"] = res.mean_exec_time_ns

    out = np.empty((1, N, 512), dtype=np.float32)
    zscale = np.float32(2.0 ** MASK_SHIFT)
    for core in range(8):
        rg, hg = divmod(core, C)
        O = res.results[core]["out"]                        # [NIC, 128, 512]
        Zz = res.results[core]["zout"]                      # [NIC, 2, 512]
        blk = O.reshape(NIC, HPC, DV, 512).transpose(0, 3, 1, 2)  # [NIC, 512, h, e]
        zb = Zz.transpose(0, 2, 1)[..., None] * zscale            # [NIC, 512, h, 1]
        out[0, rg * NQ : (rg + 1) * NQ, hg * HPC * DV : (hg + 1) * HPC * DV] = (
            (blk / zb).reshape(NQ, HPC * DV)
        )
    return out


# revision 55
# speedup vs baseline: 1.0243x; 1.0243x over previous
"""Trainium2 Bass kernel for nn_PostAttention (sparse_attention).

Computation (B=1, N=4096, H=8, d_qk=96, d_v=64):
    proj = qk @ W_qk -> q, k per head;  v = v_cls @ W_v per head
    S = q @ k.T * scale;  E = exp(S);  Z_i = sum_j E
    out_i = sum_j E_ij * m_ij * v_j / (Z_i * H * M_i),  M_i = sum_j m_ij

Sharding: 8 cores as 2 query-row groups x 4 head groups (2 heads/core).

Division of labor vs the device:
  * host pre-scales the mask rows by 2^14/(H*M_i)  -> no M column sums and
    no per-query normalization math on-chip
  * host divides the returned numerator O by (Z * 2^14) -> no PE transposes,
    no reciprocal chain; the device returns O^T ([e, i]) and Z rows as-is.

Device layout: everything transposed (S^T = [key j on partitions, query i on
free dim]) so exp output E^T / P^T feed the P@V matmul directly as the moving
operand.  exp runs on ScalarE over PAIRS of j-tiles ([128, 2*512] spanning two
PSUM banks); back-to-back ACTIVATEs pipeline to ~N/1.2GHz, making ScalarE the
pacing engine for the attention phase.  To keep it busy from the start, the
i-chunk 0 attention stream is interleaved into the projection phase (pair p
of i-chunk 0 only needs the KT/V chunks produced up to projection chunk p//2),
the Q projections for i-chunks 1-3 are deferred into the attention stream
(their qk chunks stay resident in SBUF), and one global consumer pipeline
runs across i-chunk boundaries.  Z column sums are ones-vector matmuls packed
into spare PE column groups; both heads' Z streams share one PSUM bank and
both heads' PV streams share one PSUM bank (the has_written clear from
start=True is scoped to the partition range the matmul's column groups
engage, so per-head streams with their own start=True don't disturb each
other).  PSUM budget: 2 proj accumulators + 4 (phase A) / 6 (phase B) S
banks + Z + O = 8.

dtypes: fp16 operands everywhere on the PE (1 cyc/row), fp32 PSUM accum.
(fp8 was tried and rejected: for signed contractions the per-element e4m3
quantization error does NOT average down -- measured 9.9e-2 output error.)
"""
import os
import sys

sys.path.insert(0, "/opt/trn_rl_repo")
import numpy as np

import concourse.mybir as mybir
import concourse.tile as tile
from concourse import bacc
from concourse.bass_utils import run_bass_kernel_spmd

f32 = mybir.dt.float32
f16 = mybir.dt.float16
FT = mybir.ActivationFunctionType

N = 4096
H = 8
DQK = 96
DV = 64
R = 2              # row groups
C = 4              # head groups
HPC = H // C       # heads per core = 2
NQ = N // R        # queries per core = 2048
NIC = NQ // 512    # i-chunks per core = 4
NJT = N // 128     # j tiles = 32
NPAIR = NJT // 2   # j-tile pairs = 16
SCALE = (256 // 8) ** -0.5
EXP_BIAS = -4.0    # uniform shift inside exp; cancels in the Z ratio
MASK_SHIFT = 14    # mask rows pre-scaled by 2^14/(H*M_i); host divides it out

_CACHED = {}


def _build_nc():
    nc = bacc.Bacc(name="post_attention")

    qkT = nc.declare_dram_parameter("qkT", [8, 128, 6, 512], f16, isOutput=False)
    vT = nc.declare_dram_parameter("vT", [8, 128, 4, 512], f16, isOutput=False)
    wq = nc.declare_dram_parameter("wq", [128, 6, HPC * DQK], f16, isOutput=False)
    wk = nc.declare_dram_parameter("wk", [128, 6, HPC * DQK], f16, isOutput=False)
    wv = nc.declare_dram_parameter("wv", [128, 4, HPC * DV], f16, isOutput=False)
    maskt = nc.declare_dram_parameter("maskt", [NIC, NPAIR, 128, 2, 512], f16, isOutput=False)
    # out[ic] = O^T chunk [e(2 heads x 64), i(512)]; zout[ic] = Z rows [h, i]
    out = nc.declare_dram_parameter("out", [NIC, 128, 512], f32, isOutput=True)
    zout = nc.declare_dram_parameter("zout", [NIC, HPC, 512], f32, isOutput=True)

    with tile.TileContext(nc) as tc:
        with (
            tc.tile_pool(name="const", bufs=1) as const,
            tc.tile_pool(name="persist", bufs=1) as persist,
            tc.tile_pool(name="mt", bufs=3) as mtp,
            tc.tile_pool(name="ep", bufs=3) as ep,
            tc.tile_pool(name="fin", bufs=2) as fin,
            tc.tile_pool(name="ps_z", bufs=1, space="PSUM") as ps_z,
            tc.tile_pool(name="ps_o", bufs=1, space="PSUM") as ps_o,
        ):
            ones16 = const.tile([128, 1], f16)
            nc.vector.memset(ones16, 1.0)
            bias_t = const.tile([128, 1], f32)
            nc.vector.memset(bias_t, EXP_BIAS)

            QT = persist.tile([DQK, HPC, NQ], f16)
            KT = persist.tile([DQK, HPC, N], f16)
            V = persist.tile([128, NJT, HPC * DV], f16)
            # qk chunks 1..3 stay resident so the Q projections for
            # i-chunks 1..3 can run in phase B (hidden under the exp pace)
            # instead of lengthening the PE-bound phase A.
            qkeep = persist.tile([128, 3, 6, 512], f16)

            # One compiled kernel serves all cores: the host rolls the N axis
            # of qkT / vT / mask columns so this core's query rows sit at
            # columns [0, NQ); the j-sum is order-invariant.

            def attn_unit(ic, jt0, njt, s_tile_fn):
                """Emit mask DMA + S matmuls + exp + mask-mul for j-tiles
                jt0..jt0+njt-1 of i-chunk ic.  Returns [(e_t, p_t)] per
                head.  Bigger njt amortizes the ~190-cycle fixed cost of
                each ACTIVATE on the pacing engine."""
                icol = slice(ic * 512, (ic + 1) * 512)
                mt = mtp.tile([128, njt, 512], f16, tag="mt", name="mt")
                nc.sync.dma_start(out=mt, in_=maskt[ic, jt0 // 2])
                cur = []
                for h in range(HPC):
                    s_ps = s_tile_fn(h, njt)
                    for t in range(njt):
                        jrow = slice((jt0 + t) * 128, (jt0 + t + 1) * 128)
                        nc.tensor.matmul(
                            s_ps[:, t, :],
                            lhsT=KT[:, h, jrow],
                            rhs=QT[:, h, icol],
                        )
                    e_t = ep.tile([128, njt, 512], f16, tag=f"e{h}", name=f"e_t{h}")
                    nc.scalar.activation(e_t, s_ps, FT.Exp, bias=bias_t, scale=SCALE)
                    p_t = ep.tile([128, njt, 512], f16, tag=f"p{h}", name=f"p_t{h}")
                    nc.vector.tensor_mul(p_t, e_t, mt)
                    cur.append((e_t, p_t))
                return cur

            def attn_consumers(jt0, njt, prev, z_ps, o_ps):
                """Z and PV matmuls for one unit (inputs `prev` from
                attn_unit).  Both heads' streams share one Z bank / one O
                bank; col-group pairs run concurrently on the PE.
                (fp8 DoubleRow can't be used here: it requires the matmul
                dst to start at partition 0, which breaks the col-group
                packing that makes the two heads concurrent.)"""
                for t in range(njt):
                    first = jt0 + t == 0
                    last = jt0 + t == NJT - 1
                    for h in range(HPC):
                        nc.tensor.matmul(
                            z_ps[32 * h : 32 * h + 1, :],
                            lhsT=ones16,
                            rhs=prev[h][0][:, t, :],
                            start=first,
                            stop=last,
                            tile_position=(0, 32 * h),
                            skip_group_check=True,
                        )
                for t in range(njt):
                    jt = jt0 + t
                    first = jt == 0
                    last = jt == NJT - 1
                    for h in range(HPC):
                        nc.tensor.matmul(
                            o_ps[64 * h : 64 * (h + 1), :],
                            lhsT=V[:, jt, 64 * h : 64 * (h + 1)],
                            rhs=prev[h][1][:, t, :],
                            start=first,
                            stop=last,
                            tile_position=(0, 64 * h),
                            skip_group_check=True,
                        )

            def drain_ic(ic, z_ps, o_ps):
                # Z first: the next i-chunk's consumers hit the Z bank
                # before the O bank.  The out DMA splits 4 ways so its
                # 2KB-per-partition descriptors spread across queues.
                z_sb = fin.tile([33, 512], f32, tag="zsb", name="z_sb")
                for h in range(HPC):
                    nc.vector.tensor_copy(z_sb[32 * h : 32 * h + 1, :], z_ps[32 * h : 32 * h + 1, :])
                    nc.sync.dma_start(out=zout[ic, h : h + 1, :], in_=z_sb[32 * h : 32 * h + 1, :])
                o_sb = fin.tile([128, 512], f32, tag="osb", name="o_sb")
                nc.vector.tensor_copy(o_sb, o_ps)
                for q in range(4):
                    sl = slice(32 * q, 32 * (q + 1))
                    nc.sync.dma_start(out=out[ic, sl], in_=o_sb[sl])

            # One global software pipeline across all (ic, unit) tiles:
            # consumers lag the S/exp/mul stream by two units, INCLUDING
            # across i-chunk boundaries, so the exp engine never drains while
            # the PE runs an epilogue.
            zo = {}
            pending = []  # [(ic, jt0, njt, cur), ...] awaiting consumers

            def push_unit(ic, jt0, njt, cur):
                pending.append((ic, jt0, njt, cur))
                if len(pending) > 2:
                    flush_unit()
                # Flush an i-chunk's LAST unit eagerly (lag 1 instead of 2):
                # its drain copies then enter the VectorE queue before the
                # next chunk's mask-muls, so the next chunk's first Z/PV
                # don't stall ~2us on the bank WAR.
                while pending and pending[0][1] + pending[0][2] == NJT:
                    flush_unit()

            def flush_unit():
                icj, jt0, njt, prev = pending.pop(0)
                attn_consumers(jt0, njt, prev, *zo[icj])
                if jt0 + njt == NJT:
                    drain_ic(icj, *zo[icj])

            # ---- phase A: projection with i-chunk 0 attention interleaved ----
            # PSUM: proj accumulators 2 banks (rotating tag) + S pairs for
            # ic0 2x2 banks (tag per head) + Z 1 + O 1 = 8.
            zo[0] = (
                ps_z.tile([33, 512], f32, tag="z", name="z_ps0"),
                ps_o.tile([128, 512], f32, tag="o", name="o_ps0"),
            )
            with (
                tc.tile_pool(name="wpool", bufs=1) as wpool,
                tc.tile_pool(name="qs", bufs=3) as qs,
                tc.tile_pool(name="pp", bufs=2, space="PSUM") as pp,
                tc.tile_pool(name="ps_sa", bufs=1, space="PSUM") as ps_sa,
            ):
                wq_t = persist.tile([128, 6, HPC * DQK], f16)  # outlives phase A
                nc.sync.dma_start(out=wq_t, in_=wq[:, :, :])
                wk_t = wpool.tile([128, 6, HPC * DQK], f16)
                nc.sync.dma_start(out=wk_t, in_=wk[:, :, :])
                wv_t = wpool.tile([128, 4, HPC * DV], f16)
                nc.sync.dma_start(out=wv_t, in_=wv[:, :, :])

                np0 = 0  # pairs emitted for ic0

                def ic0_step():
                    nonlocal np0
                    cur = attn_unit(
                        0, 2 * np0, 2,
                        lambda h, njt: ps_sa.tile([128, njt, 512], f32, tag=f"s{h}", name=f"sa_ps{h}"),
                    )
                    push_unit(0, 2 * np0, 2, cur)
                    np0 += 1

                for n in range(8):
                    ncol = slice(n * 512, (n + 1) * 512)
                    if 1 <= n <= 3:
                        qk_sl = qkeep[:, n - 1]
                    else:
                        qk_sl = qs.tile([128, 6, 512], f16, tag="qksl", name="qk_sl")
                    # host pre-tiles qkT/vT per chunk: each DMA is one
                    # contiguous 3KB/2KB line per partition (descriptor-rate
                    # bound otherwise).  Chunk 0 gates the kernel start, so
                    # it splits per-c across six queues.
                    if n == 0:
                        for c in range(6):
                            nc.sync.dma_start(out=qk_sl[:, c, :], in_=qkT[n, :, c, :])
                    else:
                        nc.sync.dma_start(out=qk_sl[:, 0:3, :], in_=qkT[n, :, 0:3, :])
                        nc.sync.dma_start(out=qk_sl[:, 3:6, :], in_=qkT[n, :, 3:6, :])
                    v_sl = qs.tile([128, 4, 512], f16, tag="vsl")
                    nc.sync.dma_start(out=v_sl, in_=vT[n])

                    for h in range(HPC):
                        kt_ps = pp.tile([DQK, 512], f32, tag="pj", name=f"kt_ps{h}")
                        for c in range(6):
                            nc.tensor.matmul(
                                kt_ps,
                                lhsT=wk_t[:, c, h * DQK : (h + 1) * DQK],
                                rhs=qk_sl[:, c, :],
                                start=(c == 0),
                                stop=(c == 5),
                            )
                        nc.vector.tensor_copy(KT[:, h, ncol], kt_ps)

                    if n == 0:  # query rows live in columns [0, NQ) after host roll
                        for h in range(HPC):
                            qt_ps = pp.tile([DQK, 512], f32, tag="pj", name=f"qt_ps{h}")
                            for c in range(6):
                                nc.tensor.matmul(
                                    qt_ps,
                                    lhsT=wq_t[:, c, h * DQK : (h + 1) * DQK],
                                    rhs=qk_sl[:, c, :],
                                    start=(c == 0),
                                    stop=(c == 5),
                                )
                            nc.vector.tensor_copy(QT[:, h, ncol], qt_ps)

                    # V directly in [j, e] layout: contraction over channel
                    # chunks with vT slices as the stationary operand.
                    for jj in range(4):
                        jt = 4 * n + jj
                        vj_ps = pp.tile([128, HPC * DV], f32, tag="pj", name="vj_ps")
                        for c in range(4):
                            nc.tensor.matmul(
                                vj_ps,
                                lhsT=v_sl[:, c, jj * 128 : (jj + 1) * 128],
                                rhs=wv_t[:, c, :],
                                start=(c == 0),
                                stop=(c == 3),
                            )
                        nc.vector.tensor_copy(V[:, jt, :], vj_ps)

                    # interleave i-chunk 0 attention: pairs 2n, 2n+1 only
                    # need KT/V chunks <= n (with the depth-2 consumer lag).
                    if n >= 1:
                        ic0_step()
                        ic0_step()

                def qproj(icq, qt_tile_fn):
                    """Q projection for i-chunk icq, emitted where the PE has
                    slack (the exp engine is saturated with backlog); the
                    PSUM accumulator borrows a slot of the local S rotation."""
                    icolq = slice(icq * 512, (icq + 1) * 512)
                    for h in range(HPC):
                        qt_ps = qt_tile_fn(h)
                        for c in range(6):
                            nc.tensor.matmul(
                                qt_ps,
                                lhsT=wq_t[:, c, h * DQK : (h + 1) * DQK],
                                rhs=qkeep[:, icq - 1, c, :],
                                start=(c == 0),
                                stop=(c == 5),
                            )
                        nc.vector.tensor_copy(QT[:, h, icolq], qt_ps)

                # pp's banks are idle once chunk 7 is done; the exp engine
                # still has a pair backlog here, so the PE has slack.
                qproj(1, lambda h: pp.tile([DQK, 512], f32, tag="pj", name=f"qt_ps{h}"))
                while np0 < NPAIR:
                    ic0_step()

            # ---- phase B: attention i-chunks 1..3 ----
            # ic0's last two consumer groups flush inside the global loop,
            # after phase B's first S units, so ScalarE never idles at the
            # phase seam or at i-chunk boundaries.  Each i-chunk's Q
            # projection runs here too (hidden under the exp pace), its PSUM
            # accumulator borrowing a slot of the S pool rotation.
            with tc.tile_pool(name="ps_sb", bufs=3, space="PSUM") as ps_sb:
                for ic in range(1, NIC):
                    zo[ic] = (
                        ps_z.tile([33, 512], f32, tag="z", name="z_ps"),
                        ps_o.tile([128, 512], f32, tag="o", name="o_ps"),
                    )
                    for p in range(NPAIR):
                        if p == 8 and ic < NIC - 1:
                            qproj(ic + 1, lambda h: ps_sb.tile([DQK, 512], f32, tag="s", name=f"qt_ps{h}"))
                        cur = attn_unit(
                            ic, 2 * p, 2,
                            lambda h, njt: ps_sb.tile([128, njt, 512], f32, tag="s", name=f"s_ps{h}"),
                        )
                        push_unit(ic, 2 * p, 2, cur)
                while pending:
                    flush_unit()

    nc.finalize()
    return nc


def kernel(**inputs) -> np.ndarray:
    qk = np.asarray(inputs["qk"], dtype=np.float32)        # [1, N, 768]
    v_cls = np.asarray(inputs["v_cls"], dtype=np.float32)  # [1, N, 512]
    masks = np.asarray(inputs["masks"], dtype=np.float32)  # [1, N, N]
    W_qk = np.asarray(inputs["W_qk"], dtype=np.float32)    # [768, 1536]
    W_v = np.asarray(inputs["W_v"], dtype=np.float32)      # [512, 512]

    if "nc" not in _CACHED:
        _CACHED["nc"] = _build_nc()
    nc = _CACHED["nc"]

    # Pre-scale mask rows by 2^14/(H*M_i): folds the H and mask-sum
    # normalizations into the mask; the host divides the 2^14 back out.
    m0 = masks[0]
    row_w = (2.0 ** MASK_SHIFT) / (H * m0.sum(axis=1))
    mask_scaled = (m0 * row_w[:, None]).astype(np.float16)
    # Roll the key/value axis per row group so each core's query rows start at
    # column 0; the kernel reads Q from columns [0, NQ) and pairs KT j-tiles
    # with identically rolled mask columns, so the j-sum is just reordered.
    # All device inputs are pre-tiled so every DMA reads one contiguous
    # line per partition (the DMA engines are descriptor-rate bound).
    qkT_rg, vT_rg, mask_rg = [], [], []
    for rg in range(R):
        h0 = rg * NQ
        qk_roll = np.roll(qk[0], -h0, axis=0)
        v_roll = np.roll(v_cls[0], -h0, axis=0)
        qkT_rg.append(np.ascontiguousarray(
            qk_roll.T.astype(np.float16).reshape(6, 128, 8, 512).transpose(2, 1, 0, 3)))
        vT_rg.append(np.ascontiguousarray(
            v_roll.T.astype(np.float16).reshape(4, 128, 8, 512).transpose(2, 1, 0, 3)))
        mT = np.roll(mask_scaled[h0 : h0 + NQ], -h0, axis=1).T
        mask_rg.append(np.ascontiguousarray(
            mT.reshape(NPAIR, 2, 128, NIC, 512).transpose(3, 0, 2, 1, 4)))
    wq_hg, wk_hg, wv_hg = [], [], []
    for hg in range(C):
        hs = hg * HPC
        wq_hg.append(np.ascontiguousarray(
            W_qk[:, hs * DQK : (hs + HPC) * DQK].astype(np.float16).reshape(6, 128, HPC * DQK).transpose(1, 0, 2)))
        wk_hg.append(np.ascontiguousarray(
            W_qk[:, 768 + hs * DQK : 768 + (hs + HPC) * DQK].astype(np.float16).reshape(6, 128, HPC * DQK).transpose(1, 0, 2)))
        wv_hg.append(np.ascontiguousarray(
            W_v[:, hs * DV : (hs + HPC) * DV].astype(np.float16).reshape(4, 128, HPC * DV).transpose(1, 0, 2)))
    in_maps = []
    for core in range(8):
        rg, hg = divmod(core, C)
        in_maps.append({
            "qkT": qkT_rg[rg],
            "vT": vT_rg[rg],
            "wq": wq_hg[hg],
            "wk": wk_hg[hg],
            "wv": wv_hg[hg],
            "maskt": mask_rg[rg],
        })

    trace = os.environ.get("KERNEL_TRACE", "0") == "1"
    res = run_bass_kernel_spmd(nc, in_maps, list(range(8)), trace=trace)
    if trace:
        _CACHED["exec_time_ns"] = res.exec_time_ns
        _CACHED["mean_exec_time_ns"] = res.mean_exec_time_ns

    out = np.empty((1, N, 512), dtype=np.float32)
    zscale = np.float32(2.0 ** MASK_SHIFT)
    for core in range(8):
        rg, hg = divmod(core, C)
        O = res.results[core]["out"]                        # [NIC, 128, 512]
        Zz = res.results[core]["zout"]                      # [NIC, 2, 512]
        blk = O.reshape(NIC, HPC, DV, 512).transpose(0, 3, 1, 2)  # [NIC, 512, h, e]
        zb = Zz.transpose(0, 2, 1)[..., None] * zscale            # [NIC, 512, h, 1]
        out[0, rg * NQ : (rg + 1) * NQ, hg * HPC * DV : (hg + 1) * HPC * DV] = (
            (blk / zb).reshape(NQ, HPC * DV)
        )
    return out
